# revision 21
# baseline (speedup 1.0000x reference)
"""Trainium2 Bass kernel for causal self-attention with GQA + RoPE.

Model: B=2, T=2048, C=2048, H=16 query heads, H_KV=4 kv heads, D=128.

Sharding (8 NeuronCores, pure SPMD, no collectives):
  core i -> batch b = i // 4, kv-group g = i % 4
            (query heads 4g..4g+3, kv head g, all T positions of batch b).
  o_proj partial sums (tensor-parallel all-reduce) are added on the host.

v3 schedule: three dense phases, PE kept gap-free so the tensor engine
stays at its high p-state.

  Phase 1  projections + rope. First 512-token chunk runs cc-major
           (all 6 outputs per contraction step) so the PE consumes the
           xt/weight DMA stream as it lands; later chunks run
           head-major. The rope rot-matmul / V transposes are issued
           with a lag so the PE never waits on ACT evacuations.
  Phase 2  causal attention, software-pipelined S-pair -> exp -> PV.
           P tiles are bf16: the PV/rowsum matmuls run at full rate and
           the DVE mask/add ops get 2x throughput. The softmax
           denominator quad-accumulates P on DVE, so the ones-matmul
           runs once per 4 subtiles instead of per subtile (-28us PE).
           The rowsum/rinv/scale chain is issued 2 S-pairs late to give
           DVE slack.
  Phase 3  o_proj: per 128-row q block x 512-col chunk, 4-deep psum
           pipeline, evacuations alternate DVE/ACT, wo resident
           (loaded once during phase 2).
"""

import math
import os

import numpy as np

os.environ.setdefault("MYCRO_LOCAL_CACHE", "1")

P = 128
D = 128
H = 16
H_KV = 4
GQ = H // H_KV  # 4 query heads per kv head (= per core)
B = 2
T_FULL = 2048
C_DIM = 2048
NCORES = 8
ROPE_BASE = 10000.0


def _rope_tables(T):
    inv_freq = 1.0 / (ROPE_BASE ** (np.arange(0, D, 2, dtype=np.float32) / D))
    t = np.arange(T, dtype=np.float32)
    freqs = np.outer(t, inv_freq)  # [T, D/2]
    emb = np.concatenate((freqs, freqs), axis=-1)  # [T, D]
    return (
        np.ascontiguousarray(np.cos(emb).T.astype(np.float32)),  # [D, T]
        np.ascontiguousarray(np.sin(emb).T.astype(np.float32)),
    )


def _rot_lhsT():
    # rotate_half(q) = R @ q with R[d, d+64] = -1 (d < 64), R[d, d-64] = +1.
    # matmul computes lhsT.T @ rhs, so pass lhsT = R^T.
    R = np.zeros((D, D), dtype=np.float32)
    half = D // 2
    R[np.arange(half), np.arange(half) + half] = -1.0
    R[np.arange(half) + half, np.arange(half)] = 1.0
    return np.ascontiguousarray(R.T)


def _mask4():
    # mask4[m][k, q] = 1 if (128*m + k) <= q else 0, for the 4 diagonal
    # k-subtiles of a 512-wide q chunk (S^T layout: k on partitions).
    m4 = np.zeros((4, P, 512), dtype=np.float32)
    q = np.arange(512)
    k = np.arange(P)
    for m in range(4):
        m4[m] = ((128 * m + k)[:, None] <= q[None, :]).astype(np.float32)
    return m4


def build_nc(T=T_FULL):
    """Build the per-core Bass/Tile program (identical across cores)."""
    from contextlib import ExitStack

    import concourse.mybir as mybir
    import concourse.tile as tile
    from concourse import bacc
    from concourse.masks import make_identity

    f32 = mybir.dt.float32
    f32r = mybir.dt.float32r
    bf16 = mybir.dt.bfloat16
    Exp = mybir.ActivationFunctionType.Exp
    MULT = mybir.AluOpType.mult
    ADD = mybir.AluOpType.add
    SCALE = 1.0 / math.sqrt(D)

    NCC = C_DIM // P  # 16 contraction chunks
    NQC = T // 512  # projection / attention q-chunks (512-wide)
    NCT = C_DIM // 512  # o_proj column tiles
    NKB = T // P  # 128-wide k subtiles
    XG = 4  # xt c-chunks per streamed tile

    nc = bacc.Bacc(
        "TRN2",
        target_bir_lowering=False,
        debug=False,
        num_devices=NCORES,
    )

    # x and the weights stream in as bf16: phase 1 is HBM-bus-bound at
    # the start, and bf16 x+w costs only ~3e-3 max rel err (CPU-sim'd)
    xt = nc.dram_tensor("xt", [P, NCC * T], bf16, kind="ExternalInput").ap()
    wq = nc.dram_tensor("wq", [C_DIM, GQ * D], bf16, kind="ExternalInput").ap()
    # wk/wv host-prearranged to [P, NCC*D] so DMAs are contiguous
    # per-partition descriptors (strided 512B ones choke the SW DGE)
    wk = nc.dram_tensor("wk", [P, NCC * D], bf16, kind="ExternalInput").ap()
    wv = nc.dram_tensor("wv", [P, NCC * D], bf16, kind="ExternalInput").ap()
    wo = nc.dram_tensor("wo", [GQ * D, C_DIM], bf16, kind="ExternalInput").ap()
    cosT = nc.dram_tensor("cosT", [D, T], f32, kind="ExternalInput").ap()
    sinT = nc.dram_tensor("sinT", [D, T], f32, kind="ExternalInput").ap()
    mask4 = nc.dram_tensor("mask4", [4, P, 512], bf16, kind="ExternalInput").ap()
    onesm = nc.dram_tensor("onesm", [P, P], bf16, kind="ExternalInput").ap()
    rotm = nc.dram_tensor("rotm", [P, P], f32r, kind="ExternalInput").ap()
    out = nc.dram_tensor("out", [T, C_DIM], f32, kind="ExternalOutput").ap()

    with tile.TileContext(nc) as tc, ExitStack() as ctx:
        const = ctx.enter_context(tc.tile_pool(name="const", bufs=1))
        acts = ctx.enter_context(tc.tile_pool(name="acts", bufs=1))

        wq_r = wq.rearrange("(cc p) n -> p cc n", p=P)
        xt_r = xt.rearrange("p (cc t) -> p cc t", cc=NCC)
        wo_r = wo.rearrange("(h p) n -> p h n", p=P)

        ones_sb = const.tile([P, P], bf16)
        rot_sb = const.tile([P, P], f32r)
        ident = const.tile([P, P], bf16)

        # long-lived activations (phase 1 -> 2)
        qt_sb = [acts.tile([P, T], f32r, name=f"qt{h}") for h in range(GQ)]
        kt_sb = acts.tile([P, T], f32r, name="kt")
        v_sb = acts.tile([P, NKB, D], bf16, name="vnat")

        # ---------------- phase 1: projections + rope ----------------
        with (
            tc.tile_pool(name="pwts", bufs=1) as wpool,
            tc.tile_pool(name="xts", bufs=1) as xt_pool,
            tc.tile_pool(name="rope_t", bufs=1) as rope_pool,
            tc.tile_pool(name="proj_ps", bufs=1, space="PSUM") as proj_ps,
            tc.tile_pool(name="aux_ps", bufs=1, space="PSUM") as aux_ps,
            tc.tile_pool(name="ptmp", bufs=1) as ptmp,
        ):
            wq_sb = wpool.tile([P, NCC, GQ * D], bf16)
            wk_sb = wpool.tile([P, NCC, D], bf16)
            wv_sb = wpool.tile([P, NCC, D], bf16)
            cos_sb = rope_pool.tile([P, T], f32)
            sin_sb = rope_pool.tile([P, T], f32)

            # --- DMA lead-in.  sync: xt (resident, host-packed so each
            # partition reads 4KB-contiguous rows; first cc lands in ~2us);
            # scalar: wq per-cc; gpsimd: kv, then consts.
            xt_sb = xt_pool.tile([P, NCC, T], bf16, name="xt_sb")
            nc.sync.dma_start(xt_sb[:, 0:1, :], xt_r[:, 0:1, :])
            nc.sync.dma_start(xt_sb[:, 1:4, :], xt_r[:, 1:4, :])
            for xg in range(1, NCC // XG):
                nc.sync.dma_start(
                    xt_sb[:, xg * XG : (xg + 1) * XG, :],
                    xt_r[:, xg * XG : (xg + 1) * XG, :],
                )
            for cc in range(NCC):
                nc.scalar.dma_start(wq_sb[:, cc, :], wq_r[:, cc, :])
            nc.gpsimd.dma_start(wk_sb[:, 0:XG, :], wk[:, 0 : XG * D])
            nc.gpsimd.dma_start(wv_sb[:, 0:XG, :], wv[:, 0 : XG * D])
            nc.gpsimd.dma_start(wk_sb[:, XG:, :], wk[:, XG * D :])
            nc.gpsimd.dma_start(wv_sb[:, XG:, :], wv[:, XG * D :])
            nc.gpsimd.dma_start(ones_sb[:], onesm)
            nc.gpsimd.dma_start(rot_sb[:], rotm)
            nc.gpsimd.dma_start(cos_sb[:], cosT)
            nc.gpsimd.dma_start(sin_sb[:], sinT)
            make_identity(nc, ident)

            # deferred PE tails, issued with a lag so the PE never waits
            # on the ACT evacuations feeding them
            pending = []

            def wsl_for(w):
                if w < GQ:
                    return wq_sb[:, :, w * D : (w + 1) * D]
                return wk_sb if w == GQ else wv_sb

            def rope_tail(pt_ps, dst, q0):
                # dst = pt*cos + (R pt)*sin ; raw is the ACT evacuation
                raw = ptmp.tile([P, 512], f32r, name="rraw", tag="rraw", bufs=8)
                nc.scalar.copy(raw[:], pt_ps[:])
                cosq = cos_sb[:, q0 : q0 + 512]
                sinq = sin_sb[:, q0 : q0 + 512]

                def issue_pe():
                    rp = aux_ps.tile([P, 512], f32, name="rotp", tag="rotp")
                    nc.tensor.matmul(rp[:], rot_sb[:], raw[:], start=True, stop=True)
                    nc.vector.tensor_tensor(dst, raw[:], cosq, MULT)
                    t2 = ptmp.tile([P, 512], f32, name="rt2", tag="rt2", bufs=2)
                    nc.vector.tensor_tensor(t2[:], rp[:], sinq, MULT)
                    nc.vector.tensor_tensor(dst, dst, t2[:], ADD)

                pending.append(issue_pe)

            def v_tail(vp_ps, qc):
                vraw = ptmp.tile([P, 512], bf16, name="vraw", tag="vraw", bufs=2)
                nc.scalar.copy(vraw[:], vp_ps[:])

                def issue_pe():
                    # 4 PE transposes into ONE psum tile, single DVE evac
                    tp = aux_ps.tile([P, 512], bf16, name="vtrp", tag="vtrp")
                    for ks in range(4):
                        nc.tensor.transpose(
                            tp[:, ks * P : (ks + 1) * P],
                            vraw[:, ks * P : (ks + 1) * P],
                            ident[:],
                        )
                    nc.vector.tensor_copy(v_sb[:, qc * 4 : qc * 4 + 4, :], tp[:])

                pending.append(issue_pe)

            def finish_run(pp, w, qc, q0):
                if w < GQ:
                    rope_tail(pp, qt_sb[w][:, q0 : q0 + 512], q0)
                elif w == GQ:
                    rope_tail(pp, kt_sb[:, q0 : q0 + 512], q0)
                else:
                    v_tail(pp, qc)

            # ---- qc 0: cc-major so the PE tracks the DMA stream
            def xtile0(cc):
                return xt_sb[:, cc, 0:512]

            pps = {}
            for w in range(6):
                pps[w] = proj_ps.tile([P, 512], f32, name=f"pp{w}", tag=f"pp{w}")
            for cc in range(NCC):
                for w in range(6):
                    nc.tensor.matmul(
                        pps[w][:],
                        wsl_for(w)[:, cc, :],
                        xtile0(cc),
                        start=(cc == 0),
                        stop=(cc == NCC - 1),
                    )
            for w in range(6):
                finish_run(pps[w], w, 0, 0)

            # ---- qc 1..3: head-major runs; tails pop one per run
            for qc in range(1, NQC):
                q0 = qc * 512
                for w in range(6):
                    pp = proj_ps.tile([P, 512], f32, name=f"pp{w}", tag=f"pp{w}")
                    wsl = wsl_for(w)
                    for cc in range(NCC):
                        nc.tensor.matmul(
                            pp[:],
                            wsl[:, cc, :],
                            xt_sb[:, cc, q0 : q0 + 512],
                            start=(cc == 0),
                            stop=(cc == NCC - 1),
                        )
                    finish_run(pp, w, qc, q0)
                    if len(pending) > 1:
                        pending.pop(0)()
            while pending:
                pending.pop(0)()

        # ------------- phases 2+3: attention, then o_proj -------------
        with tc.tile_pool(name="acts2", bufs=1) as acts2:
            y_sb = [acts2.tile([P, T], bf16, name=f"yt{h}") for h in range(GQ)]
            wo_sb = acts2.tile([P, GQ, C_DIM], bf16, name="wo_sb")
            mask_sb = acts2.tile([P, 4, 512], bf16, name="mask_sb")
            for m in range(4):
                nc.gpsimd.dma_start(mask_sb[:, m, :], mask4[m])
            for h in range(GQ):
                nc.gpsimd.dma_start(wo_sb[:, h, :], wo_r[:, h, :])

            attn_scope = ExitStack()
            pt_pool = attn_scope.enter_context(tc.tile_pool(name="pt_pool", bufs=1))
            s_ps = attn_scope.enter_context(
                tc.tile_pool(name="s_ps", bufs=1, space="PSUM")
            )
            y_ps = attn_scope.enter_context(
                tc.tile_pool(name="y_ps", bufs=1, space="PSUM")
            )
            nrm_pool = attn_scope.enter_context(tc.tile_pool(name="nrm", bufs=1))

            # Flat software-pipelined chunk stream across all (aq, h)
            # blocks: the S matmuls of chunk i+1 are always issued before
            # the exp/mask/add/PV of chunk i, including across block
            # boundaries, so the ACT exp stream never starves. The
            # diagonal 512x512 block is split into 256-wide q halves
            # (fp32r stays full-rate at free>=256) to skip ~1/4 of the
    # wasted causal work.
            schedule = []  # (s_fn, post_fn)
            norm_at = {}  # chunk index -> norm chain fn
            state = {}  # per-block shared tiles, keyed (aq, h)

            def mk_block(aq, h):
                q0 = aq * 512
                nks = 4 * aq + 4
                npn = 2 * aq  # normal (non-diagonal) pairs
                st = {"accs": [], "yp": None}
                state[(aq, h)] = st
                qrhs = qt_sb[h]

                def get_yp():
                    if st["yp"] is None:
                        st["yp"] = y_ps.tile([P, 512], f32, name="yp", tag="yp", bufs=2)
                    return st["yp"]

                def mm_y(lhs, rhs, col0, ncol, start, stop):
                    nc.tensor.matmul(
                        get_yp()[:, col0 : col0 + ncol],
                        lhs,
                        rhs,
                        start=start,
                        stop=stop,
                        skip_group_check=True,
                    )

                # --- normal pairs
                def mk_norm_pair(g):
                    def s_fn(slot):
                        sp = s_ps.tile([P, 1024], f32, name="sp", tag="sp", bufs=3)
                        for j in range(2):
                            ks = 2 * g + j
                            nc.tensor.matmul(
                                sp[:, j * 512 : (j + 1) * 512],
                                kt_sb[:, ks * P : (ks + 1) * P],
                                qrhs[:, q0 : q0 + 512],
                                start=True,
                                stop=True,
                            )
                        slot["sp"] = sp

                    def post_fn(slot):
                        pt = pt_pool.tile(
                            [P, 1024], bf16, name="ptile", tag="ptile", bufs=4
                        )
                        nc.scalar.activation(pt[:], slot["sp"][:], Exp, scale=SCALE)
                        ptL, ptR = pt[:, 0:512], pt[:, 512:1024]
                        if g % 2 == 0:
                            acc = nrm_pool.tile(
                                [P, 512], bf16, name="acc", tag="acc", bufs=6
                            )
                            nc.vector.tensor_tensor(acc[:], ptL, ptR, ADD)
                            st["accs"].append(acc)
                        else:
                            acc = st["accs"][-1]
                            nc.vector.tensor_tensor(acc[:], acc[:], ptL, ADD)
                            nc.vector.tensor_tensor(acc[:], acc[:], ptR, ADD)
                        for j in range(2):
                            ks = 2 * g + j
                            mm_y(
                                v_sb[:, ks, :],
                                pt[:, j * 512 : (j + 1) * 512],
                                0,
                                512,
                                start=(ks == 0),
                                stop=False,
                            )

                    return s_fn, post_fn

                for g in range(npn):
                    schedule.append(mk_block_item(mk_norm_pair(g)))

                # --- diagonal block, q-half 0 (local q 0..255): k-subs 0,1
                def dh0_s(slot):
                    sp = s_ps.tile([P, 1024], f32, name="sp", tag="sp", bufs=3)
                    for j in range(2):
                        ks = 4 * aq + j
                        nc.tensor.matmul(
                            sp[:, j * 256 : (j + 1) * 256],
                            kt_sb[:, ks * P : (ks + 1) * P],
                            qrhs[:, q0 : q0 + 256],
                            start=True,
                            stop=True,
                        )
                    slot["sp"] = sp

                def dh0_post(slot):
                    pt = pt_pool.tile(
                        [P, 512], bf16, name="ptile5", tag="ptile", bufs=4
                    )
                    nc.scalar.activation(pt[:], slot["sp"][:, 0:512], Exp, scale=SCALE)
                    # m0: triangle on first 128 cols; m1: full 256 cols
                    nc.gpsimd.tensor_tensor(
                        pt[:, 0:128], pt[:, 0:128], mask_sb[:, 0, 0:128], MULT
                    )
                    nc.gpsimd.tensor_tensor(
                        pt[:, 256:512], pt[:, 256:512], mask_sb[:, 1, 0:256], MULT
                    )
                    acc = nrm_pool.tile([P, 512], bf16, name="accd", tag="acc", bufs=6)
                    st["accs"].append(acc)
                    nc.vector.tensor_tensor(
                        acc[:, 0:256], pt[:, 0:256], pt[:, 256:512], ADD
                    )
                    first = aq == 0
                    mm_y(v_sb[:, 4 * aq, :], pt[:, 0:256], 0, 256, first, False)
                    mm_y(v_sb[:, 4 * aq + 1, :], pt[:, 256:512], 0, 256, False, True)

                schedule.append(mk_block_item((dh0_s, dh0_post)))

                # --- diagonal block, q-half 1 (local q 256..511): k-subs 0-3
                def dh1_s(slot):
                    sp = s_ps.tile([P, 1024], f32, name="sp", tag="sp", bufs=3)
                    for j in range(4):
                        ks = 4 * aq + j
                        nc.tensor.matmul(
                            sp[:, j * 256 : (j + 1) * 256],
                            kt_sb[:, ks * P : (ks + 1) * P],
                            qrhs[:, q0 + 256 : q0 + 512],
                            start=True,
                            stop=True,
                        )
                    slot["sp"] = sp

                def dh1_post(slot):
                    pt = pt_pool.tile(
                        [P, 1024], bf16, name="ptile6", tag="ptile", bufs=4
                    )
                    nc.scalar.activation(pt[:], slot["sp"][:], Exp, scale=SCALE)
                    # m2: triangle on its first 128 cols; m3: full 256 cols
                    nc.gpsimd.tensor_tensor(
                        pt[:, 512:640], pt[:, 512:640], mask_sb[:, 2, 256:384], MULT
                    )
                    nc.gpsimd.tensor_tensor(
                        pt[:, 768:1024], pt[:, 768:1024], mask_sb[:, 3, 256:512], MULT
                    )
                    acc = st["accs"][-1]
                    nc.vector.tensor_tensor(
                        acc[:, 256:512], pt[:, 0:256], pt[:, 256:512], ADD
                    )
                    nc.vector.tensor_tensor(
                        acc[:, 256:512], acc[:, 256:512], pt[:, 512:768], ADD
                    )
                    nc.vector.tensor_tensor(
                        acc[:, 256:512], acc[:, 256:512], pt[:, 768:1024], ADD
                    )
                    first = aq == 0
                    for j in range(4):
                        mm_y(
                            v_sb[:, 4 * aq + j, :],
                            pt[:, j * 256 : (j + 1) * 256],
                            256,
                            256,
                            first and j == 0,
                            j == 3,
                        )

                schedule.append(mk_block_item((dh1_s, dh1_post)))

                def norm_fn():
                    # rowsum psum comes from the sp lane (frees a bank so
                    # the S pipeline can run with bufs=3 / lookahead 2)
                    rsp = s_ps.tile([P, 1024], f32, name="rsp", tag="sp", bufs=3)
                    rsp = rsp[:, 0:512]
                    accs = st["accs"]
                    for qd, acc in enumerate(accs):
                        nc.tensor.matmul(
                            rsp[:],
                            ones_sb[:],
                            acc[:],
                            start=(qd == 0),
                            stop=(qd == len(accs) - 1),
                        )
                    rinv = nrm_pool.tile([P, 512], f32, name="rinv", tag="rinv", bufs=2)
                    nc.vector.reciprocal_approx_fast(rinv[:], rsp[:])
                    nc.vector.tensor_tensor(
                        y_sb[h][:, q0 : q0 + 512], get_yp()[:], rinv[:], MULT
                    )

                norm_at[len(schedule) - 1] = norm_fn

            def mk_block_item(fns):
                s_fn, post_fn = fns
                slot = {}
                return (lambda: s_fn(slot), lambda: post_fn(slot))

            for aq in range(NQC):
                for h in range(GQ):
                    mk_block(aq, h)

            # run the pipeline: S(i+1) before post(i); norm chain of a
            # finished block goes right after the next block's first S
            schedule[0][0]()
            schedule[1][0]()
            for i in range(len(schedule)):
                if i + 2 < len(schedule):
                    schedule[i + 2][0]()
                if i - 1 in norm_at:
                    norm_at[i - 1]()
                schedule[i][1]()
            norm_at[len(schedule) - 1]()

            # ---------------- phase 3: o_proj ----------------
            attn_scope.close()
            with (
                tc.tile_pool(name="o_ps", bufs=1, space="PSUM") as o_ps,
                tc.tile_pool(name="ost", bufs=1) as ost_pool,
            ):
                qi = 0
                for qb in range(T // P):
                    for ct in range(NCT):
                        op = o_ps.tile([P, 512], f32, name="op", tag="op", bufs=4)
                        for h in range(GQ):
                            nc.tensor.matmul(
                                op[:],
                                y_sb[h][:, qb * P : (qb + 1) * P],
                                wo_sb[:, h, ct * 512 : (ct + 1) * 512],
                                start=(h == 0),
                                stop=(h == GQ - 1),
                            )
                        ot = ost_pool.tile([P, 512], f32, name="ot", tag="ot", bufs=6)
                        ev = nc.vector if qi % 2 == 0 else nc.scalar
                        if qi % 2 == 0:
                            ev.tensor_copy(ot[:], op[:])
                        else:
                            ev.copy(ot[:], op[:])
                        oq = (nc.gpsimd, nc.sync)[qi % 2]
                        oq.dma_start(
                            out[qb * P : (qb + 1) * P, ct * 512 : (ct + 1) * 512],
                            ot[:],
                        )
                        qi += 1

    nc.compile()
    return nc


def make_in_maps(x, wq, wk, wv, wo, T=T_FULL):
    """Per-core input dicts for run_bass_kernel_spmd."""
    import ml_dtypes

    bf16 = ml_dtypes.bfloat16
    cosT, sinT = _rope_tables(T)
    m4 = _mask4().astype(bf16)
    onesm = np.ones((P, P), dtype=bf16)
    rotm = _rot_lhsT()

    xts = [
        np.ascontiguousarray(
            x[b].T.reshape(16, P, T).transpose(1, 0, 2).reshape(P, 16 * T).astype(bf16)
        )
        for b in range(B)
    ]
    in_maps = []
    for core in range(NCORES):
        b, g = core // 4, core % 4
        in_maps.append(
            {
                "xt": xts[b],
                "wq": np.ascontiguousarray(wq[:, 512 * g : 512 * (g + 1)].astype(bf16)),
                "wk": np.ascontiguousarray(
                    wk[:, D * g : D * (g + 1)]
                    .reshape(16, P, D)
                    .transpose(1, 0, 2)
                    .reshape(P, 16 * D)
                    .astype(bf16)
                ),
                "wv": np.ascontiguousarray(
                    wv[:, D * g : D * (g + 1)]
                    .reshape(16, P, D)
                    .transpose(1, 0, 2)
                    .reshape(P, 16 * D)
                    .astype(bf16)
                ),
                "wo": np.ascontiguousarray(wo[512 * g : 512 * (g + 1), :].astype(bf16)),
                "cosT": cosT,
                "sinT": sinT,
                "mask4": m4,
                "onesm": onesm,
                "rotm": rotm,
            }
        )
    return in_maps


_NC_CACHE = {}


def _get_nc(T=T_FULL):
    if T not in _NC_CACHE:
        _NC_CACHE[T] = build_nc(T)
    return _NC_CACHE[T]


def run(inputs, trace=False):
    """Run on 8 NeuronCores. Returns (full_output, BassKernelResults)."""
    from concourse.bass_utils import run_bass_kernel_spmd

    x = np.asarray(inputs["x"], dtype=np.float32)
    in_maps = make_in_maps(
        x,
        np.asarray(inputs["wq"], dtype=np.float32),
        np.asarray(inputs["wk"], dtype=np.float32),
        np.asarray(inputs["wv"], dtype=np.float32),
        np.asarray(inputs["wo"], dtype=np.float32),
    )
    nc = _get_nc()
    res = run_bass_kernel_spmd(nc, in_maps, list(range(NCORES)), trace=trace)
    outs = res.results
    full = np.zeros((B, T_FULL, C_DIM), dtype=np.float32)
    for core in range(NCORES):
        full[core // 4] += outs[core]["out"]
    return full, res


def kernel(**inputs):
    full, _ = run(inputs, trace=False)
    return full


# revision 22
# speedup vs baseline: 1.2066x; 1.2066x over previous
"""Trainium2 Bass kernel for causal self-attention with GQA + RoPE.

Model: B=2, T=2048, C=2048, H=16 query heads, H_KV=4 kv heads, D=128.

Sharding (8 NeuronCores, pure SPMD, no collectives):
  core i -> batch b = i // 4, kv-group g = i % 4
            (query heads 4g..4g+3, kv head g, all T positions of batch b).
  o_proj partial sums (tensor-parallel all-reduce) are added on the host.

v3 schedule: three dense phases, PE kept gap-free so the tensor engine
stays at its high p-state.

  Phase 1  projections + rope. First 512-token chunk runs cc-major
           (all 6 outputs per contraction step) so the PE consumes the
           xt/weight DMA stream as it lands; later chunks run
           head-major. The rope rot-matmul / V transposes are issued
           with a lag so the PE never waits on ACT evacuations.
  Phase 2  causal attention, software-pipelined S-pair -> exp -> PV.
           P tiles are bf16: the PV/rowsum matmuls run at full rate and
           the DVE mask/add ops get 2x throughput. The softmax
           denominator quad-accumulates P on DVE, so the ones-matmul
           runs once per 4 subtiles instead of per subtile (-28us PE).
           The rowsum/rinv/scale chain is issued 2 S-pairs late to give
           DVE slack.
  Phase 3  o_proj: per 128-row q block x 512-col chunk, 4-deep psum
           pipeline, evacuations alternate DVE/ACT, wo resident
           (loaded once during phase 2).
"""

import math
import os

import numpy as np

os.environ.setdefault("MYCRO_LOCAL_CACHE", "1")

P = 128
D = 128
H = 16
H_KV = 4
GQ = H // H_KV  # 4 query heads per kv head (= per core)
B = 2
T_FULL = 2048
C_DIM = 2048
NCORES = 8
ROPE_BASE = 10000.0


def _rope_tables(T):
    inv_freq = 1.0 / (ROPE_BASE ** (np.arange(0, D, 2, dtype=np.float32) / D))
    t = np.arange(T, dtype=np.float32)
    freqs = np.outer(t, inv_freq)  # [T, D/2]
    emb = np.concatenate((freqs, freqs), axis=-1)  # [T, D]
    return (
        np.ascontiguousarray(np.cos(emb).T.astype(np.float32)),  # [D, T]
        np.ascontiguousarray(np.sin(emb).T.astype(np.float32)),
    )


def _rot_lhsT():
    # rotate_half(q) = R @ q with R[d, d+64] = -1 (d < 64), R[d, d-64] = +1.
    # matmul computes lhsT.T @ rhs, so pass lhsT = R^T.
    R = np.zeros((D, D), dtype=np.float32)
    half = D // 2
    R[np.arange(half), np.arange(half) + half] = -1.0
    R[np.arange(half) + half, np.arange(half)] = 1.0
    return np.ascontiguousarray(R.T)


def _mask4():
    # mask4[m][k, q] = 1 if (128*m + k) <= q else 0, for the 4 diagonal
    # k-subtiles of a 512-wide q chunk (S^T layout: k on partitions).
    m4 = np.zeros((4, P, 512), dtype=np.float32)
    q = np.arange(512)
    k = np.arange(P)
    for m in range(4):
        m4[m] = ((128 * m + k)[:, None] <= q[None, :]).astype(np.float32)
    return m4


def build_nc(T=T_FULL):
    """Build the per-core Bass/Tile program (identical across cores)."""
    from contextlib import ExitStack

    import concourse.mybir as mybir
    import concourse.tile as tile
    from concourse import bacc
    from concourse.masks import make_identity

    f32 = mybir.dt.float32
    f32r = mybir.dt.float32r
    bf16 = mybir.dt.bfloat16
    Exp = mybir.ActivationFunctionType.Exp
    MULT = mybir.AluOpType.mult
    ADD = mybir.AluOpType.add
    SCALE = 1.0 / math.sqrt(D)

    NCC = C_DIM // P  # 16 contraction chunks
    NQC = T // 512  # projection / attention q-chunks (512-wide)
    NCT = C_DIM // 512  # o_proj column tiles
    NKB = T // P  # 128-wide k subtiles
    XG = 4  # xt c-chunks per streamed tile

    nc = bacc.Bacc(
        "TRN2",
        target_bir_lowering=False,
        debug=False,
        num_devices=NCORES,
    )

    # x and the weights stream in as bf16: phase 1 is HBM-bus-bound at
    # the start, and bf16 x+w costs only ~3e-3 max rel err (CPU-sim'd)
    xt = nc.dram_tensor("xt", [P, NCC * T], bf16, kind="ExternalInput").ap()
    wq = nc.dram_tensor("wq", [C_DIM, GQ * D], bf16, kind="ExternalInput").ap()
    # wk/wv host-prearranged to [P, NCC*D] so DMAs are contiguous
    # per-partition descriptors (strided 512B ones choke the SW DGE)
    wk = nc.dram_tensor("wk", [P, NCC * D], bf16, kind="ExternalInput").ap()
    wv = nc.dram_tensor("wv", [P, NCC * D], bf16, kind="ExternalInput").ap()
    wo = nc.dram_tensor("wo", [GQ * D, C_DIM], bf16, kind="ExternalInput").ap()
    cosT = nc.dram_tensor("cosT", [D, T], f32, kind="ExternalInput").ap()
    sinT = nc.dram_tensor("sinT", [D, T], f32, kind="ExternalInput").ap()
    mask4 = nc.dram_tensor("mask4", [4, P, 512], bf16, kind="ExternalInput").ap()
    onesm = nc.dram_tensor("onesm", [P, P], bf16, kind="ExternalInput").ap()
    rotm = nc.dram_tensor("rotm", [P, P], f32r, kind="ExternalInput").ap()
    out = nc.dram_tensor("out", [T, C_DIM], f32, kind="ExternalOutput").ap()

    with tile.TileContext(nc) as tc, ExitStack() as ctx:
        const = ctx.enter_context(tc.tile_pool(name="const", bufs=1))
        acts = ctx.enter_context(tc.tile_pool(name="acts", bufs=1))

        wq_r = wq.rearrange("(cc p) n -> p cc n", p=P)
        xt_r = xt.rearrange("p (cc t) -> p cc t", cc=NCC)
        wo_r = wo.rearrange("(h p) n -> p h n", p=P)

        ones_sb = const.tile([P, P], bf16)
        rot_sb = const.tile([P, P], f32r)
        ident = const.tile([P, P], bf16)

        # long-lived activations (phase 1 -> 2)
        qt_sb = [acts.tile([P, T], f32r, name=f"qt{h}") for h in range(GQ)]
        kt_sb = acts.tile([P, T], f32r, name="kt")
        v_sb = acts.tile([P, NKB, D], bf16, name="vnat")

        # ---------------- phase 1: projections + rope ----------------
        with (
            tc.tile_pool(name="pwts", bufs=1) as wpool,
            tc.tile_pool(name="xts", bufs=1) as xt_pool,
            tc.tile_pool(name="rope_t", bufs=1) as rope_pool,
            tc.tile_pool(name="proj_ps", bufs=1, space="PSUM") as proj_ps,
            tc.tile_pool(name="aux_ps", bufs=1, space="PSUM") as aux_ps,
            tc.tile_pool(name="ptmp", bufs=1) as ptmp,
        ):
            wq_sb = wpool.tile([P, NCC, GQ * D], bf16)
            wk_sb = wpool.tile([P, NCC, D], bf16)
            wv_sb = wpool.tile([P, NCC, D], bf16)
            cos_sb = rope_pool.tile([P, T], f32)
            sin_sb = rope_pool.tile([P, T], f32)

            # --- DMA lead-in.  sync: xt (resident, host-packed so each
            # partition reads 4KB-contiguous rows; first cc lands in ~2us);
            # scalar: wq per-cc; gpsimd: kv, then consts.
            xt_sb = xt_pool.tile([P, NCC, T], bf16, name="xt_sb")
            nc.sync.dma_start(xt_sb[:, 0:1, :], xt_r[:, 0:1, :])
            nc.sync.dma_start(xt_sb[:, 1:4, :], xt_r[:, 1:4, :])
            for xg in range(1, NCC // XG):
                nc.sync.dma_start(
                    xt_sb[:, xg * XG : (xg + 1) * XG, :],
                    xt_r[:, xg * XG : (xg + 1) * XG, :],
                )
            for cc in range(NCC):
                nc.scalar.dma_start(wq_sb[:, cc, :], wq_r[:, cc, :])
            nc.gpsimd.dma_start(wk_sb[:, 0:XG, :], wk[:, 0 : XG * D])
            nc.gpsimd.dma_start(wv_sb[:, 0:XG, :], wv[:, 0 : XG * D])
            nc.gpsimd.dma_start(wk_sb[:, XG:, :], wk[:, XG * D :])
            nc.gpsimd.dma_start(wv_sb[:, XG:, :], wv[:, XG * D :])
            nc.gpsimd.dma_start(ones_sb[:], onesm)
            nc.gpsimd.dma_start(rot_sb[:], rotm)
            nc.gpsimd.dma_start(cos_sb[:], cosT)
            nc.gpsimd.dma_start(sin_sb[:], sinT)
            make_identity(nc, ident)

            # deferred PE tails, issued with a lag so the PE never waits
            # on the ACT evacuations feeding them
            pending = []

            def wsl_for(w):
                if w < GQ:
                    return wq_sb[:, :, w * D : (w + 1) * D]
                return wk_sb if w == GQ else wv_sb

            def rope_tail(pt_ps, dst, q0):
                # dst = pt*cos + (R pt)*sin ; raw is the ACT evacuation
                raw = ptmp.tile([P, 512], f32r, name="rraw", tag="rraw", bufs=8)
                nc.scalar.copy(raw[:], pt_ps[:])
                cosq = cos_sb[:, q0 : q0 + 512]
                sinq = sin_sb[:, q0 : q0 + 512]

                def issue_pe():
                    rp = aux_ps.tile([P, 512], f32, name="rotp", tag="rotp")
                    nc.tensor.matmul(rp[:], rot_sb[:], raw[:], start=True, stop=True)
                    nc.vector.tensor_tensor(dst, raw[:], cosq, MULT)
                    t2 = ptmp.tile([P, 512], f32, name="rt2", tag="rt2", bufs=2)
                    nc.vector.tensor_tensor(t2[:], rp[:], sinq, MULT)
                    nc.vector.tensor_tensor(dst, dst, t2[:], ADD)

                pending.append(issue_pe)

            def v_tail(vp_ps, qc):
                vraw = ptmp.tile([P, 512], bf16, name="vraw", tag="vraw", bufs=2)
                nc.scalar.copy(vraw[:], vp_ps[:])

                def issue_pe():
                    # 4 PE transposes into ONE psum tile, single DVE evac
                    tp = aux_ps.tile([P, 512], bf16, name="vtrp", tag="vtrp")
                    for ks in range(4):
                        nc.tensor.transpose(
                            tp[:, ks * P : (ks + 1) * P],
                            vraw[:, ks * P : (ks + 1) * P],
                            ident[:],
                        )
                    nc.vector.tensor_copy(v_sb[:, qc * 4 : qc * 4 + 4, :], tp[:])

                pending.append(issue_pe)

            def finish_run(pp, w, qc, q0):
                if w < GQ:
                    rope_tail(pp, qt_sb[w][:, q0 : q0 + 512], q0)
                elif w == GQ:
                    rope_tail(pp, kt_sb[:, q0 : q0 + 512], q0)
                else:
                    v_tail(pp, qc)

            # ---- qc 0: cc-major so the PE tracks the DMA stream
            def xtile0(cc):
                return xt_sb[:, cc, 0:512]

            pps = {}
            for w in range(6):
                pps[w] = proj_ps.tile([P, 512], f32, name=f"pp{w}", tag=f"pp{w}")
            for cc in range(NCC):
                for w in range(6):
                    nc.tensor.matmul(
                        pps[w][:],
                        wsl_for(w)[:, cc, :],
                        xtile0(cc),
                        start=(cc == 0),
                        stop=(cc == NCC - 1),
                    )
            for w in range(6):
                finish_run(pps[w], w, 0, 0)

            # ---- qc 1..3: head-major runs; tails pop one per run
            for qc in range(1, NQC):
                q0 = qc * 512
                for w in range(6):
                    pp = proj_ps.tile([P, 512], f32, name=f"pp{w}", tag=f"pp{w}")
                    wsl = wsl_for(w)
                    for cc in range(NCC):
                        nc.tensor.matmul(
                            pp[:],
                            wsl[:, cc, :],
                            xt_sb[:, cc, q0 : q0 + 512],
                            start=(cc == 0),
                            stop=(cc == NCC - 1),
                        )
                    finish_run(pp, w, qc, q0)
                    if len(pending) > 1:
                        pending.pop(0)()
            while pending:
                pending.pop(0)()

        # ------------- phases 2+3: attention, then o_proj -------------
        with tc.tile_pool(name="acts2", bufs=1) as acts2:
            y_sb = [acts2.tile([P, T], bf16, name=f"yt{h}") for h in range(GQ)]
            wo_sb = acts2.tile([P, GQ, C_DIM], bf16, name="wo_sb")
            mask_sb = acts2.tile([P, 4, 512], bf16, name="mask_sb")
            for m in range(4):
                nc.gpsimd.dma_start(mask_sb[:, m, :], mask4[m])
            for h in range(GQ):
                nc.gpsimd.dma_start(wo_sb[:, h, :], wo_r[:, h, :])

            attn_scope = ExitStack()
            pt_pool = attn_scope.enter_context(tc.tile_pool(name="pt_pool", bufs=1))
            s_ps = attn_scope.enter_context(
                tc.tile_pool(name="s_ps", bufs=1, space="PSUM")
            )
            y_ps = attn_scope.enter_context(
                tc.tile_pool(name="y_ps", bufs=1, space="PSUM")
            )
            nrm_pool = attn_scope.enter_context(tc.tile_pool(name="nrm", bufs=1))

            # Flat software-pipelined chunk stream across all (aq, h)
            # blocks: the S matmuls of chunk i+1 are always issued before
            # the exp/mask/add/PV of chunk i, including across block
            # boundaries, so the ACT exp stream never starves. The
            # diagonal 512x512 block is split into 256-wide q halves
            # (fp32r stays full-rate at free>=256) to skip ~1/4 of the
    # wasted causal work.
            schedule = []  # (s_fn, post_fn)
            norm_at = {}  # chunk index -> norm chain fn
            state = {}  # per-block shared tiles, keyed (aq, h)

            def mk_block(aq, h):
                q0 = aq * 512
                nks = 4 * aq + 4
                npn = 2 * aq  # normal (non-diagonal) pairs
                st = {"accs": [], "yp": None}
                state[(aq, h)] = st
                qrhs = qt_sb[h]

                def get_yp():
                    if st["yp"] is None:
                        st["yp"] = y_ps.tile([P, 512], f32, name="yp", tag="yp", bufs=2)
                    return st["yp"]

                def mm_y(lhs, rhs, col0, ncol, start, stop):
                    nc.tensor.matmul(
                        get_yp()[:, col0 : col0 + ncol],
                        lhs,
                        rhs,
                        start=start,
                        stop=stop,
                        skip_group_check=True,
                    )

                # --- normal pairs
                def mk_norm_pair(g):
                    def s_fn(slot):
                        sp = s_ps.tile([P, 1024], f32, name="sp", tag="sp", bufs=3)
                        for j in range(2):
                            ks = 2 * g + j
                            nc.tensor.matmul(
                                sp[:, j * 512 : (j + 1) * 512],
                                kt_sb[:, ks * P : (ks + 1) * P],
                                qrhs[:, q0 : q0 + 512],
                                start=True,
                                stop=True,
                            )
                        slot["sp"] = sp

                    def post_fn(slot):
                        pt = pt_pool.tile(
                            [P, 1024], bf16, name="ptile", tag="ptile", bufs=4
                        )
                        nc.scalar.activation(pt[:], slot["sp"][:], Exp, scale=SCALE)
                        ptL, ptR = pt[:, 0:512], pt[:, 512:1024]
                        if g % 2 == 0:
                            acc = nrm_pool.tile(
                                [P, 512], bf16, name="acc", tag="acc", bufs=6
                            )
                            nc.vector.tensor_tensor(acc[:], ptL, ptR, ADD)
                            st["accs"].append(acc)
                        else:
                            acc = st["accs"][-1]
                            nc.vector.tensor_tensor(acc[:], acc[:], ptL, ADD)
                            nc.vector.tensor_tensor(acc[:], acc[:], ptR, ADD)
                        for j in range(2):
                            ks = 2 * g + j
                            mm_y(
                                v_sb[:, ks, :],
                                pt[:, j * 512 : (j + 1) * 512],
                                0,
                                512,
                                start=(ks == 0),
                                stop=False,
                            )

                    return s_fn, post_fn

                for g in range(npn):
                    schedule.append(mk_block_item(mk_norm_pair(g)))

                # --- diagonal block, q-half 0 (local q 0..255): k-subs 0,1
                def dh0_s(slot):
                    sp = s_ps.tile([P, 1024], f32, name="sp", tag="sp", bufs=3)
                    for j in range(2):
                        ks = 4 * aq + j
                        nc.tensor.matmul(
                            sp[:, j * 256 : (j + 1) * 256],
                            kt_sb[:, ks * P : (ks + 1) * P],
                            qrhs[:, q0 : q0 + 256],
                            start=True,
                            stop=True,
                        )
                    slot["sp"] = sp

                def dh0_post(slot):
                    pt = pt_pool.tile(
                        [P, 512], bf16, name="ptile5", tag="ptile", bufs=4
                    )
                    nc.scalar.activation(pt[:], slot["sp"][:, 0:512], Exp, scale=SCALE)
                    # m0: triangle on first 128 cols; m1: full 256 cols
                    nc.vector.tensor_tensor(
                        pt[:, 0:128], pt[:, 0:128], mask_sb[:, 0, 0:128], MULT
                    )
                    nc.vector.tensor_tensor(
                        pt[:, 256:512], pt[:, 256:512], mask_sb[:, 1, 0:256], MULT
                    )
                    acc = nrm_pool.tile([P, 512], bf16, name="accd", tag="acc", bufs=6)
                    st["accs"].append(acc)
                    nc.vector.tensor_tensor(
                        acc[:, 0:256], pt[:, 0:256], pt[:, 256:512], ADD
                    )
                    first = aq == 0
                    mm_y(v_sb[:, 4 * aq, :], pt[:, 0:256], 0, 256, first, False)
                    mm_y(v_sb[:, 4 * aq + 1, :], pt[:, 256:512], 0, 256, False, True)

                schedule.append(mk_block_item((dh0_s, dh0_post)))

                # --- diagonal block, q-half 1 (local q 256..511): k-subs 0-3
                def dh1_s(slot):
                    sp = s_ps.tile([P, 1024], f32, name="sp", tag="sp", bufs=3)
                    for j in range(4):
                        ks = 4 * aq + j
                        nc.tensor.matmul(
                            sp[:, j * 256 : (j + 1) * 256],
                            kt_sb[:, ks * P : (ks + 1) * P],
                            qrhs[:, q0 + 256 : q0 + 512],
                            start=True,
                            stop=True,
                        )
                    slot["sp"] = sp

                def dh1_post(slot):
                    pt = pt_pool.tile(
                        [P, 1024], bf16, name="ptile6", tag="ptile", bufs=4
                    )
                    nc.scalar.activation(pt[:], slot["sp"][:], Exp, scale=SCALE)
                    # m2: triangle on its first 128 cols; m3: full 256 cols
                    nc.vector.tensor_tensor(
                        pt[:, 512:640], pt[:, 512:640], mask_sb[:, 2, 256:384], MULT
                    )
                    nc.vector.tensor_tensor(
                        pt[:, 768:1024], pt[:, 768:1024], mask_sb[:, 3, 256:512], MULT
                    )
                    acc = st["accs"][-1]
                    nc.vector.tensor_tensor(
                        acc[:, 256:512], pt[:, 0:256], pt[:, 256:512], ADD
                    )
                    nc.vector.tensor_tensor(
                        acc[:, 256:512], acc[:, 256:512], pt[:, 512:768], ADD
                    )
                    nc.vector.tensor_tensor(
                        acc[:, 256:512], acc[:, 256:512], pt[:, 768:1024], ADD
                    )
                    first = aq == 0
                    for j in range(4):
                        mm_y(
                            v_sb[:, 4 * aq + j, :],
                            pt[:, j * 256 : (j + 1) * 256],
                            256,
                            256,
                            first and j == 0,
                            j == 3,
                        )

                schedule.append(mk_block_item((dh1_s, dh1_post)))

                def norm_fn():
                    # rowsum psum comes from the sp lane (frees a bank so
                    # the S pipeline can run with bufs=3 / lookahead 2)
                    rsp = s_ps.tile([P, 1024], f32, name="rsp", tag="sp", bufs=3)
                    rsp = rsp[:, 0:512]
                    accs = st["accs"]
                    for qd, acc in enumerate(accs):
                        nc.tensor.matmul(
                            rsp[:],
                            ones_sb[:],
                            acc[:],
                            start=(qd == 0),
                            stop=(qd == len(accs) - 1),
                        )
                    rinv = nrm_pool.tile([P, 512], f32, name="rinv", tag="rinv", bufs=2)
                    nc.vector.reciprocal_approx_fast(rinv[:], rsp[:])
                    nc.vector.tensor_tensor(
                        y_sb[h][:, q0 : q0 + 512], get_yp()[:], rinv[:], MULT
                    )

                norm_at[len(schedule) - 1] = norm_fn

            def mk_block_item(fns):
                s_fn, post_fn = fns
                slot = {}
                return (lambda: s_fn(slot), lambda: post_fn(slot))

            for aq in range(NQC):
                for h in range(GQ):
                    mk_block(aq, h)

            # run the pipeline: S(i+1) before post(i); norm chain of a
            # finished block goes right after the next block's first S
            schedule[0][0]()
            schedule[1][0]()
            for i in range(len(schedule)):
                if i + 2 < len(schedule):
                    schedule[i + 2][0]()
                if i - 1 in norm_at:
                    norm_at[i - 1]()
                schedule[i][1]()
            norm_at[len(schedule) - 1]()

            # ---------------- phase 3: o_proj ----------------
            attn_scope.close()
            with (
                tc.tile_pool(name="o_ps", bufs=1, space="PSUM") as o_ps,
                tc.tile_pool(name="ost", bufs=1) as ost_pool,
            ):
                qi = 0
                for qb in range(T // P):
                    for ct in range(NCT):
                        op = o_ps.tile([P, 512], f32, name="op", tag="op", bufs=4)
                        for h in range(GQ):
                            nc.tensor.matmul(
                                op[:],
                                y_sb[h][:, qb * P : (qb + 1) * P],
                                wo_sb[:, h, ct * 512 : (ct + 1) * 512],
                                start=(h == 0),
                                stop=(h == GQ - 1),
                            )
                        ot = ost_pool.tile([P, 512], f32, name="ot", tag="ot", bufs=6)
                        ev = nc.vector if qi % 2 == 0 else nc.scalar
                        if qi % 2 == 0:
                            ev.tensor_copy(ot[:], op[:])
                        else:
                            ev.copy(ot[:], op[:])
                        oq = (nc.gpsimd, nc.sync)[qi % 2]
                        oq.dma_start(
                            out[qb * P : (qb + 1) * P, ct * 512 : (ct + 1) * 512],
                            ot[:],
                        )
                        qi += 1

    nc.compile()
    return nc


def make_in_maps(x, wq, wk, wv, wo, T=T_FULL):
    """Per-core input dicts for run_bass_kernel_spmd."""
    import ml_dtypes

    bf16 = ml_dtypes.bfloat16
    cosT, sinT = _rope_tables(T)
    m4 = _mask4().astype(bf16)
    onesm = np.ones((P, P), dtype=bf16)
    rotm = _rot_lhsT()

    xts = [
        np.ascontiguousarray(
            x[b].T.reshape(16, P, T).transpose(1, 0, 2).reshape(P, 16 * T).astype(bf16)
        )
        for b in range(B)
    ]
    in_maps = []
    for core in range(NCORES):
        b, g = core // 4, core % 4
        in_maps.append(
            {
                "xt": xts[b],
                "wq": np.ascontiguousarray(wq[:, 512 * g : 512 * (g + 1)].astype(bf16)),
                "wk": np.ascontiguousarray(
                    wk[:, D * g : D * (g + 1)]
                    .reshape(16, P, D)
                    .transpose(1, 0, 2)
                    .reshape(P, 16 * D)
                    .astype(bf16)
                ),
                "wv": np.ascontiguousarray(
                    wv[:, D * g : D * (g + 1)]
                    .reshape(16, P, D)
                    .transpose(1, 0, 2)
                    .reshape(P, 16 * D)
                    .astype(bf16)
                ),
                "wo": np.ascontiguousarray(wo[512 * g : 512 * (g + 1), :].astype(bf16)),
                "cosT": cosT,
                "sinT": sinT,
                "mask4": m4,
                "onesm": onesm,
                "rotm": rotm,
            }
        )
    return in_maps


_NC_CACHE = {}


def _get_nc(T=T_FULL):
    if T not in _NC_CACHE:
        _NC_CACHE[T] = build_nc(T)
    return _NC_CACHE[T]


def run(inputs, trace=False):
    """Run on 8 NeuronCores. Returns (full_output, BassKernelResults)."""
    from concourse.bass_utils import run_bass_kernel_spmd

    x = np.asarray(inputs["x"], dtype=np.float32)
    in_maps = make_in_maps(
        x,
        np.asarray(inputs["wq"], dtype=np.float32),
        np.asarray(inputs["wk"], dtype=np.float32),
        np.asarray(inputs["wv"], dtype=np.float32),
        np.asarray(inputs["wo"], dtype=np.float32),
    )
    nc = _get_nc()
    res = run_bass_kernel_spmd(nc, in_maps, list(range(NCORES)), trace=trace)
    outs = res.results
    full = np.zeros((B, T_FULL, C_DIM), dtype=np.float32)
    for core in range(NCORES):
        full[core // 4] += outs[core]["out"]
    return full, res


def kernel(**inputs):
    full, _ = run(inputs, trace=False)
    return full
